# revision 49
# baseline (speedup 1.0000x reference)
"""Expert-choice MoE kernel for 8 Trainium2 NeuronCores (Bacc/Tile).

Distribution: expert-parallel, one expert per core.
  - gate: each core computes fp32 scores z = x_shard @ Wg for its 1/8 token
    shard (pre-swizzled so the wrapped-16 view loads contiguously later),
    AllToAll -> each core holds the full (N,) score row of ITS expert.
  - top-k (k=2048 of N=8192): exact fp32 threshold via a parallel-64
    candidate search (each partition holds HALF the logits; one
    tensor_scalar/accum count per round + a fold matmul merges the
    halves; 5 rounds of 64x narrowing from [-2,2) = 3.7e-9 resolution
    < fp32 ulp at the threshold), then ONE gpsimd sparse_gather
    compaction of fp32-packed (idx*2048 + 2*q10(gate)+1) values, exact
    in 24 bits (pad columns absorb score ties at the threshold).
  - dispatch: ONE dma_gather(transpose=True) per 512-token group pulls the
    selected rows from HBM already transposed to [h, tok] bf16 layout.
  - expert FFN in bf16 (fp32 accumulation), erf-Gelu on the scalar engine,
    fp32 gate multiply on the bf16 output. W2 runs fc-outer so each hid
    stationary serves two 512-wide matmuls.
  - combine: ONE dma_scatter_add (SDMA CCE add) per group into a zeroed
    bf16 (N, H) dense buffer, ReduceScatter (add, bf16) across the 8
    cores; y is emitted bf16 and upcast on the host.

Built on Bacc (not raw Bass): Bacc.compile() runs insert_library_loads
and codegen_inst_isa_subclasses, which this walrus build needs to accept
the sparse_gather/dma_gather/dma_scatter_add Pool-ucode instructions.
"""

import sys

for _p in ("/opt/trn_rl_repo",):
    if _p not in sys.path:
        sys.path.insert(0, _p)

import numpy as np
import ml_dtypes

import concourse.bass as bass
import concourse.bacc as bacc
import concourse.mybir as mybir
import concourse.tile as tile
from concourse.bass import _add_dep_helper

# ---------------------------------------------------------------------------
# Patch: this walrus build rejects >1 sync-wait on the SP Drain that
# TileContext emits at kernel exit. Split the global-clock waits across
# several drains (1 wait each).
# ---------------------------------------------------------------------------
from concourse.vector_clock import ScopedClock

_MAX_DRAIN_WAITS = 1


def _patched_drain_and_barrier(self, tick_clock, wait_clock):
    nc = self.nc
    probe = nc.sync.drain()
    wait_clock.add_sem_waits(probe.ins, ScopedClock({None: tick_clock.global_clock}))
    si = probe.ins.sync_info
    waits = list(si.on_wait or []) if si is not None else []
    if len(waits) > _MAX_DRAIN_WAITS:
        probe.ins.sync_info = mybir.SyncInfo(
            on_wait=waits[:_MAX_DRAIN_WAITS],
            on_update=list(si.on_update or []),
        )
        for i in range(_MAX_DRAIN_WAITS, len(waits), _MAX_DRAIN_WAITS):
            extra = nc.sync.drain()
            extra.ins.sync_info = mybir.SyncInfo(
                on_wait=waits[i : i + _MAX_DRAIN_WAITS], on_update=[]
            )
    nc.all_engine_barrier()
    assert self.sems is not None
    popped = nc._tile_sem_poison_stack.pop()
    assert popped is self._sem_poison
    nc.clear_and_free_semaphores(list(self.sems.allocated().values()))
    nc.all_engine_barrier()


tile.TileContext._drain_and_barrier = _patched_drain_and_barrier

_WSPLIT_LIMIT = 1
_wsplit_ctr = [0]


def _split_excess_waits(nc, limit=_WSPLIT_LIMIT):
    """This walrus build encodes at most `limit` sync-wait commands per
    instruction; hoist excess waits onto same-engine Drain instructions
    inserted immediately before (per-engine streams execute in order)."""
    f = nc.m.functions[0]
    for b in f.blocks:
        insts = b.instructions
        out = []
        changed = False
        for inst in insts:
            si = getattr(inst, "sync_info", None)
            waits = list(si.on_wait or []) if si is not None else []
            eng = getattr(inst, "engine", None)
            if len(waits) > limit and eng is not None and \
                    eng != mybir.EngineType.Unassigned:
                keep = waits[-limit:]
                extra = waits[:-limit]
                for i in range(0, len(extra), limit):
                    d = mybir.InstDrain(
                        name=f"WSPLIT-{_wsplit_ctr[0]}", ins=[], outs=[])
                    _wsplit_ctr[0] += 1
                    d.engine = eng
                    d.sync_info = mybir.SyncInfo(
                        on_wait=extra[i:i + limit], on_update=[])
                    out.append(d)
                    nc.register_instruction(d, overwrite=True)
                inst.sync_info = mybir.SyncInfo(
                    on_wait=keep, on_update=list(si.on_update or []))
                changed = True
            out.append(inst)
        if changed:
            b.instructions = out

dt = mybir.dt
Alu = mybir.AluOpType
Act = mybir.ActivationFunctionType

N_CORES = 8

FULL = dict(N=8192, H=1024, FF=4096, E=8, K=2048)


def build_moe_nc(N=8192, H=1024, FF=4096, E=8, K=2048, TOKG=512, act=None,
                 do_compile=True, skip_sg=False):
    """Build the SPMD Bacc program (same program on all 8 cores)."""
    assert E == N_CORES
    P = N // N_CORES          # tokens per shard
    HC = H // 128             # h chunks
    FC = FF // 128            # ff chunks
    NG = K // TOKG            # token groups
    SUBS = TOKG // 128        # 128-token subtiles per group
    NCOLS = K // 128          # compact cols in [128, NCOLS] layout
    W16 = N // 16             # free size of the [16, W16] wrapped layout
    K16 = K // 16             # compact cols in [16, K16] wrapped layout
    GCOLS = TOKG // 16        # idx cols consumed per group
    SGPAD = 64                # tie-absorbing pad columns for sparse_gather
    TRASH = P                 # local-combine trash row for non-local tokens
    assert K % TOKG == 0 and TOKG % 128 == 0 and P % 128 == 0
    assert K16 + SGPAD <= 512  # sparse_gather output limit
    if act is None:
        act = Act.Gelu

    nc = bacc.Bacc(None, target_bir_lowering=False, debug=False,
                   num_devices=N_CORES)

    # ---- I/O ----
    xT_s = nc.dram_tensor("xT_s", [H, P], dt.float32, kind="ExternalInput")
    x_bf = nc.dram_tensor("x_bf", [N, H], dt.bfloat16, kind="ExternalInput")
    Wg_d = nc.dram_tensor("Wg", [H, E], dt.float32, kind="ExternalInput")
    # W1/W2 uploaded HOST-PRE-SWIZZLED to the [p, c, f] SBUF layout the
    # gather-transpose contraction needs (h = c*128 + p): the device-side
    # load is then partition-contiguous — 128 big descriptors instead of
    # 1024, cutting ~40us of descriptor-gen off the sync queue.
    W1_d = nc.dram_tensor("W1", [128, HC * FF], dt.bfloat16,
                          kind="ExternalInput")
    W2_d = nc.dram_tensor("W2", [128, FC * H], dt.bfloat16,
                          kind="ExternalInput")
    b1_d = nc.dram_tensor("b1", [1, FF], dt.float32, kind="ExternalInput")
    b2_d = nc.dram_tensor("b2", [1, H], dt.float32, kind="ExternalInput")
    # per-core shard offset (c*P) for remapping global token ids to the
    # local dense buffer; [16,1] so it is a per-partition scalar operand
    # y is emitted in bf16 (the combine is bf16 anyway); the host upcasts
    y_d = nc.dram_tensor("y", [P, H], dt.bfloat16, kind="ExternalOutput")

    # ---- internal DRAM ----
    z_loc_d = nc.dram_tensor("z_loc", [E, P], dt.float32)
    z_e_d = nc.dram_tensor("z_e", [N_CORES, P], dt.float32)
    g_dram = nc.dram_tensor("g_dram", [K], dt.float32)
    dense_d = nc.dram_tensor("dense", [N, H], dt.bfloat16)
    rs_out_d = nc.dram_tensor("rs_out", [P, H], dt.bfloat16)

    groups = [list(range(N_CORES))]

    with tile.TileContext(nc) as tc:
        with (
            tc.tile_pool(name="const", bufs=1) as const_pool,
            tc.tile_pool(name="w", bufs=1) as w_pool,
            tc.tile_pool(name="psum1", bufs=2, space="PSUM") as psum1_pool,
            tc.tile_pool(name="psum2", bufs=2, space="PSUM") as psum2_pool,
        ):
            # ---------------- persistent constants ----------------
            ones1 = const_pool.tile([1, 128], dt.float32)
            nc.vector.memset(ones1[:], 1.0)
            ones64 = const_pool.tile([64, 128], dt.float32)
            nc.vector.memset(ones64[:], 1.0)

            # b2 broadcast [128, H] (constant along tokens)
            b2_sb = const_pool.tile([1, H], dt.float32)
            nc.sync.dma_start(b2_sb[:], b2_d[:])
            b2_ps = psum2_pool.tile([128, H], dt.float32, tag="ps2")
            for hh in range(0, H, 512):
                nc.tensor.matmul(b2_ps[:, hh:hh + 512], ones1[:],
                                 b2_sb[:, hh:hh + 512], start=True, stop=True)
            b2_bcast = const_pool.tile([128, H], dt.float32)
            nc.vector.tensor_copy(b2_bcast[:], b2_ps[:])

            # b1 per-partition [128, FC]
            b1_pp = const_pool.tile([128, FC], dt.float32)
            nc.sync.dma_start(
                b1_pp[:], b1_d[:].rearrange("o (c p) -> (o p) c", p=128))

            # persistent routing outputs (filled by the gate phase)
            idxs_tok = const_pool.tile([128, K16], dt.int16)
            g_pp = const_pool.tile([128, NCOLS], dt.float32)

            # ================= gate phase (scoped pool) ================
            # Emitted BEFORE the (much larger) weight DMAs so the
            # scheduler gives the latency-critical gate inputs DMA priority.
            sg_inst = None
            with (
                tc.tile_pool(name="gate", bufs=1) as gate_pool,
                tc.tile_pool(name="small", bufs=2) as small_pool,
            ):
                # h mapped as p*HC + c so each partition reads ONE
                # contiguous block per load (128 big DMA descriptors).
                # Loaded as FOUR h-quarters through a 2-buffer tag rotation
                # (16KB total; the slots are later reused by the z_bcast
                # halves) so the loads pipeline with the z matmuls; the z
                # matmul accumulates through all quarters before stopping.
                HQ = HC // 4
                wg_sb = gate_pool.tile([128, HC, E], dt.float32)
                nc.sync.dma_start(
                    wg_sb[:], Wg_d[:].rearrange("(p c) e -> p c e", p=128))

                # z_sb_loc rows are written PRE-SWIZZLED (token u stored at
                # column (u%16)*64 + u//16) so that after the AllToAll the
                # wrapped-16 [16, W16] view is a contiguous-stride load.
                z_sb_loc = gate_pool.tile([E, P], dt.float32)
                z_loc_sw = z_sb_loc[:].rearrange("e (r w) -> e w r", r=16)
                z_ps_l = []
                for t0 in range(0, P, 512):
                    z_ps = psum1_pool.tile([E, 512], dt.float32,
                                           name=f"z_ps{t0}", tag="ps1")
                    z_ps_l.append(z_ps)
                for qf in range(4):
                    xT_sb = gate_pool.tile([128, HQ, P], dt.float32,
                                           name=f"xT{qf}", tag="xT_sb",
                                           bufs=2)
                    nc.sync.dma_start(
                        xT_sb[:],
                        xT_s[:].rearrange("(p c) t -> p c t", p=128)
                        [:, qf * HQ:(qf + 1) * HQ, :])
                    for t0 in range(0, P, 512):
                        z_ps = z_ps_l[t0 // 512]
                        for ci in range(HQ):
                            nc.tensor.matmul(
                                z_ps[:], wg_sb[:, qf * HQ + ci, :],
                                xT_sb[:, ci, t0:t0 + 512],
                                start=(qf == 0 and ci == 0),
                                stop=(qf == 3 and ci == HQ - 1))
                for t0 in range(0, P, 512):
                    nc.vector.tensor_copy(
                        z_loc_sw[:, t0 // 16:(t0 + 512) // 16, :],
                        z_ps_l[t0 // 512][:])
                nc.sync.dma_start(z_loc_d[:], z_sb_loc[:])

                # core c receives every shard's scores for expert c
                nc.gpsimd.collective_compute(
                    "AllToAll", Alu.bypass, replica_groups=groups,
                    ins=[z_loc_d[:]], outs=[z_e_d[:]],
                )

                # wrapped-16 view (token j at [j%16, j//16]); contiguous
                # 64-element runs thanks to the sender-side swizzle
                # (slot shared with the now-dead z_sb_loc)
                z16 = gate_pool.tile([16, W16], dt.float32, tag="z_sb_loc")
                z16_inst = nc.sync.dma_start(
                    z16[:].rearrange("r (q w) -> r q w", q=E),
                    z_e_d[:].rearrange("q (r w) -> r q w", r=16))

                # compaction inputs that do not depend on the threshold —
                # emitted first so they overlap the A2A / bisection
                ids16 = gate_pool.tile([16, W16], dt.int32)
                nc.gpsimd.iota(ids16[:], pattern=[[16, W16]], base=0,
                               channel_multiplier=1)
                idf16 = gate_pool.tile([16, W16], dt.float32)
                nc.vector.tensor_copy(idf16[:], ids16[:])
                sig16 = gate_pool.tile([16, W16], dt.float32)
                nc.scalar.activation(sig16[:], z16[:], Act.Sigmoid)

                # ---- parallel-64 search for the k-th largest logit ----
                # Partitions p and p+64 each hold HALF of the N logits
                # (z_bcast2[p] = z16 half p//64); round r tests the 64
                # candidates base + (p%64)*step_r at once (one
                # tensor_scalar with accum_out gives per-partition partial
                # counts; partition-sliced add merges the halves).
                # base' = base + (j*-1)*step with j* = #candidates whose
                # count >= K keeps count(z >= base) >= K invariant; each
                # round shrinks the bracket 64x. Start [-2, 2) (logits are
                # N(0, ~0.64); only the THRESHOLD must lie inside the
                # bracket, and it is the 75th percentile ~= +0.43).
                # 5 rounds -> 3.7e-9 < fp32 ulp of the logits.
                # eqm[k', k*64+m] = (k==k'): selector for the broadcast
                # (iota shares the later-used scr8 slot — same 4KB)
                eqm_i = gate_pool.tile([16, 16 * 64], dt.int32, tag="scr8")
                nc.gpsimd.iota(eqm_i[:], pattern=[[1, 16], [0, 64]], base=0,
                               channel_multiplier=-1)
                eqm = gate_pool.tile([16, 16 * 64], dt.float32)
                nc.vector.tensor_scalar(eqm[:], eqm_i[:], 0, None,
                                        op0=Alu.is_equal)
                iota_p = gate_pool.tile([128, 1], dt.int32)
                nc.gpsimd.iota(iota_p[:], pattern=[[1, 1]], base=0,
                               channel_multiplier=1)
                iota_p64 = gate_pool.tile([128, 1], dt.int32)
                nc.vector.tensor_scalar(iota_p64[:], iota_p[:], 63, None,
                                        op0=Alu.bitwise_and)
                iota_pf = gate_pool.tile([128, 1], dt.float32)
                nc.vector.tensor_copy(iota_pf[:], iota_p64[:])
                # fold[p, j] = (p % 64 == j): folds the two partition
                # halves' partial counts with one matmul (a cross-base-
                # partition tensor_tensor is rejected by the verifier)
                fold_i = gate_pool.tile([128, 64], dt.int32)
                nc.gpsimd.iota(fold_i[:], pattern=[[1, 64]], base=0,
                               channel_multiplier=-1)
                nc.vector.tensor_scalar(fold_i[:], fold_i[:], 128, None,
                                        op0=Alu.add)
                nc.vector.tensor_scalar(fold_i[:], fold_i[:], 63, None,
                                        op0=Alu.bitwise_and)
                fold = gate_pool.tile([128, 64], dt.float32)
                nc.vector.tensor_scalar(fold[:], fold_i[:], 0, None,
                                        op0=Alu.is_equal)

                HW16 = W16 // 2   # 256: half of the wrapped free dim
                # two 8KB halves reusing the rotating xT slots
                zb_halves = []
                for h in range(2):
                    zbh = gate_pool.tile([128, 8, HW16], dt.float32,
                                         name=f"zb{h}", tag="xT_sb", bufs=2)
                    zb_halves.append(zbh)
                for k in range(16):
                    z_bcast = zb_halves[k // 8]
                    zb_ps = psum1_pool.tile([128, HW16], dt.float32,
                                            tag="ps1")
                    nc.tensor.matmul(zb_ps[0:64, :],
                                     eqm[:, k * 64:(k + 1) * 64],
                                     z16[:, 0:HW16],
                                     start=True, stop=True)
                    nc.tensor.matmul(zb_ps[64:128, :],
                                     eqm[:, k * 64:(k + 1) * 64],
                                     z16[:, HW16:W16],
                                     start=True, stop=True)
                    if k % 2 == 0:
                        nc.vector.tensor_copy(z_bcast[:, k % 8, :], zb_ps[:])
                    else:
                        nc.scalar.activation(z_bcast[:, k % 8, :], zb_ps[:],
                                             Act.Copy)

                kf = float(K)
                rounds = 5
                base = small_pool.tile([128, 1], dt.float32, tag="base")
                nc.vector.memset(base[:], -2.0)
                step = 4.0 / 64.0
                scr8 = gate_pool.tile([128, 8 * HW16], dt.uint8)
                for _ in range(rounds):
                    cand = small_pool.tile([128, 1], dt.float32, tag="cand")
                    nc.vector.scalar_tensor_tensor(
                        cand[:], iota_pf[:], step, base[:],
                        op0=Alu.mult, op1=Alu.add)
                    partA = small_pool.tile([128, 1], dt.float32, tag="pA")
                    nc.vector.tensor_scalar(
                        scr8[:],
                        zb_halves[0][:].rearrange("p a b -> p (a b)"),
                        cand[:, :1], None, op0=Alu.is_ge, op1=Alu.add,
                        accum_out=partA[:])
                    partB = small_pool.tile([128, 1], dt.float32, tag="pB")
                    nc.vector.tensor_scalar(
                        scr8[:],
                        zb_halves[1][:].rearrange("p a b -> p (a b)"),
                        cand[:, :1], None, op0=Alu.is_ge, op1=Alu.add,
                        accum_out=partB[:])
                    part = small_pool.tile([128, 1], dt.float32, tag="part")
                    nc.vector.tensor_tensor(part[:], partA[:], partB[:],
                                            op=Alu.add)
                    cnt_ps = psum1_pool.tile([64, 1], dt.float32, tag="ps1")
                    nc.tensor.matmul(cnt_ps[:], fold[:], part[:],
                                     start=True, stop=True)
                    geK = small_pool.tile([64, 1], dt.float32, tag="geK")
                    nc.vector.tensor_scalar(geK[:], cnt_ps[:], kf, None,
                                            op0=Alu.is_ge)
                    js_ps = psum1_pool.tile([128, 1], dt.float32, tag="ps1")
                    nc.tensor.matmul(js_ps[:], ones64[:], geK[:],
                                     start=True, stop=True)
                    # base' = fl((j*-1)*step + base) — the SAME rounding
                    # path as the tested candidate, so the count(z >= base)
                    # >= K invariant holds bit-exactly.
                    jm1 = small_pool.tile([128, 1], dt.float32, tag="jm1")
                    nc.vector.tensor_scalar(jm1[:], js_ps[:], -1.0, None,
                                            op0=Alu.add)
                    nb = small_pool.tile([128, 1], dt.float32, tag="base")
                    nc.vector.scalar_tensor_tensor(
                        nb[:], jm1[:], step, base[:],
                        op0=Alu.mult, op1=Alu.add)
                    base = nb
                    step /= 64.0
                lo = base

                # ---- selection mask + ONE packed compaction ----
                # packed = idx*2048 + (2*q10(sig) + 1): 24 bits, exact fp32.
                # Unselected slots get -1.0 (negative -> dropped by
                # sparse_gather). Output padded by SGPAD columns so fp32
                # score ties at the threshold overflow into the pad.
                sel16 = gate_pool.tile([16, W16], dt.uint8)
                nc.vector.tensor_scalar(sel16[:], z16[:], lo[:16, :1], None,
                                        op0=Alu.is_ge)
                q1 = gate_pool.tile([16, W16], dt.float32)
                nc.vector.tensor_scalar(q1[:], sig16[:], 1023.0, None,
                                        op0=Alu.mult)
                q2 = gate_pool.tile([16, W16], dt.int32)
                nc.vector.tensor_copy(q2[:], q1[:])
                nc.vector.tensor_scalar(q2[:], q2[:], 1, 1,
                                        op0=Alu.logical_shift_left,
                                        op1=Alu.bitwise_or)
                q4 = gate_pool.tile([16, W16], dt.float32, tag="ids16")
                nc.vector.tensor_copy(q4[:], q2[:])
                packed = gate_pool.tile([16, W16], dt.float32, tag="q2")
                nc.vector.scalar_tensor_tensor(
                    packed[:], idf16[:], 2048.0, q4[:],
                    op0=Alu.mult, op1=Alu.add)
                pneg = gate_pool.tile([16, W16], dt.float32, tag="q1")
                nc.vector.memset(pneg[:], -1.0)
                nc.vector.copy_predicated(pneg[:], sel16[:], packed[:])
                pc = gate_pool.tile([16, K16 + SGPAD], dt.float32)
                nf1 = gate_pool.tile([1, 1], dt.uint32)
                if not skip_sg:
                    sg_inst = nc.gpsimd.sparse_gather(pc[:], pneg[:],
                                                      num_found=nf1[:])
                else:
                    fake = gate_pool.tile([16, K16 + SGPAD], dt.int32)
                    nc.gpsimd.iota(fake[:], pattern=[[16, K16 + SGPAD]],
                                   base=0, channel_multiplier=1)
                    nc.vector.tensor_scalar(fake[:], fake[:], 11, 1025,
                                            op0=Alu.logical_shift_left,
                                            op1=Alu.bitwise_or)
                    nc.vector.tensor_copy(pc[:], fake[:])

                # unpack: idx = packed >> 11, gq = packed & 2047 (int-exact)
                pci = gate_pool.tile([16, K16], dt.int32)
                nc.vector.tensor_copy(pci[:], pc[:, :K16])
                idx_i32 = gate_pool.tile([16, K16], dt.int32)
                nc.vector.tensor_scalar(idx_i32[:], pci[:], 11, None,
                                        op0=Alu.arith_shift_right)
                idc_i16 = gate_pool.tile([16, K16], dt.int16)
                nc.vector.tensor_copy(idc_i16[:], idx_i32[:])
                gq_i32 = gate_pool.tile([16, K16], dt.int32)
                nc.vector.tensor_scalar(gq_i32[:], pci[:], 2047, None,
                                        op0=Alu.bitwise_and)
                gqf = gate_pool.tile([16, K16], dt.float32)
                nc.vector.tensor_copy(gqf[:], gq_i32[:])
                gc = gate_pool.tile([16, K16], dt.float32)
                nc.vector.tensor_scalar(gc[:], gqf[:], 1.0, 1.0 / 2046.0,
                                        op0=Alu.subtract, op1=Alu.mult)

                # idxs replicated to all 8 core blocks (for gather/scatter)
                for b in range(8):
                    nc.sync.dma_start(idxs_tok[16 * b:16 * (b + 1), :],
                                      idc_i16[:])

                # gate values to per-partition [128, NCOLS] via DRAM bounce
                nc.sync.dma_start(
                    g_dram[:].rearrange("(c r) -> r c", r=16), gc[:])
                nc.sync.dma_start(
                    g_pp[:], g_dram[:].rearrange("(q p) -> p q", p=128))

                # zero the dense combine buffer (16.8MB; each partition
                # writes contiguous 4-row (8KB) runs). SWDGE (gpsimd)
                # keeps the HWDGE rings clear for the latency-critical
                # gate-phase traffic (z store -> A2A).
                zero_sb = gate_pool.tile([128, 4, H], dt.bfloat16)
                nc.vector.memset(zero_sb[:], 0.0)
                for i in range(N // 512):
                    nc.sync.dma_start(
                        dense_d[512 * i:512 * (i + 1), :].rearrange(
                            "(p c) h -> p c h", p=128),
                        zero_sb[:])

            # ------- weights (fill DMA idle time of the gate phase;
            # emitted after it so the gate loads win priority) ---
            # Held back until the z16 load lands: 16MB of weight traffic
            # issued too early floods the SDMA rings and delays the tiny
            # gate-phase transfers (z store -> A2A -> z16) by 10-15us.
            w1_sb = w_pool.tile([128, HC, FF], dt.bfloat16)
            w1i = nc.sync.dma_start(
                w1_sb[:], W1_d[:].rearrange("p (c f) -> p c f", c=HC))
            _add_dep_helper(w1i.ins, z16_inst.ins,
                            reason="keep SDMA rings clear for gate phase")
            w2_sb = w_pool.tile([128, FC, H], dt.bfloat16)
            w2i = nc.sync.dma_start(
                w2_sb[:], W2_d[:].rearrange("p (c h) -> p c h", c=FC))
            _add_dep_helper(w2i.ins, z16_inst.ins,
                            reason="keep SDMA rings clear for gate phase")

            # ================= FFN phase ================
            with (
                tc.tile_pool(name="ex", bufs=2) as ex_pool,
                tc.tile_pool(name="hid", bufs=1) as hid_pool,
                tc.tile_pool(name="out", bufs=2) as out_pool,
            ):
                ex_tiles = {}

                def issue_gather(g):
                    ex_T = ex_pool.tile([128, HC, TOKG], dt.bfloat16,
                                        tag="ex")
                    g_inst = nc.gpsimd.dma_gather(
                        ex_T[:], x_bf[:],
                        idxs_tok[:, g * GCOLS:(g + 1) * GCOLS],
                        TOKG, TOKG, H, transpose=True)
                    if g == 0 and sg_inst is not None:
                        _add_dep_helper(g_inst.ins, sg_inst.ins,
                                        reason="keep sparse_gather lib "
                                               "phase before mlp phase")
                    ex_tiles[g] = ex_T

                issue_gather(0)
                for g in range(NG):
                    if g + 1 < NG:
                        issue_gather(g + 1)
                    ex_T = ex_tiles.pop(g)

                    hid_sb = hid_pool.tile([128, FC, TOKG], dt.bfloat16,
                                           tag="hid")
                    for fc in range(FC):
                        ps1 = psum1_pool.tile([128, TOKG], dt.float32,
                                              tag="ps1")
                        for ci in range(HC):
                            nc.tensor.matmul(
                                ps1[:], w1_sb[:, ci, fc * 128:(fc + 1) * 128],
                                ex_T[:, ci, :],
                                start=(ci == 0), stop=(ci == HC - 1))
                        nc.scalar.activation(hid_sb[:, fc, :], ps1[:], act,
                                             bias=b1_pp[:, fc:fc + 1])

                    out_sb = out_pool.tile([128, SUBS, H], dt.bfloat16,
                                           tag="osb")
                    for s in range(SUBS):
                        col = g * SUBS + s
                        pso = psum2_pool.tile([128, H], dt.float32, tag="ps2")
                        # fc-outer: each hid stationary serves both hh
                        # halves (one PE weight swap per TWO matmuls)
                        for fc in range(FC):
                            lhs = hid_sb[:, fc, s * 128:(s + 1) * 128]
                            nc.tensor.matmul(
                                pso[:, 0:512], lhs, w2_sb[:, fc, 0:512],
                                start=(fc == 0), stop=(fc == FC - 1))
                            nc.tensor.matmul(
                                pso[:, 512:1024], lhs, w2_sb[:, fc, 512:1024],
                                start=(fc == 0), stop=(fc == FC - 1))
                        nc.vector.tensor_tensor(out_sb[:, s, :], pso[:],
                                                b2_bcast[:], op=Alu.add)
                        nc.vector.tensor_scalar(out_sb[:, s, :],
                                                out_sb[:, s, :],
                                                g_pp[:, col:col + 1], None,
                                                op0=Alu.mult)
                        if g == NG - 1 and s == SUBS // 2 - 1:
                            # last group: scatter the first token-half now
                            # so its ~12us DMA data phase hides under the
                            # remaining W2 compute and the ReduceScatter
                            # triggers earlier
                            nc.gpsimd.dma_scatter_add(
                                dense_d[:], out_sb[:, 0:SUBS // 2, :],
                                idxs_tok[:, g * GCOLS:
                                         g * GCOLS + GCOLS // 2],
                                TOKG // 2, TOKG // 2, H)

                    if g == NG - 1:
                        nc.gpsimd.dma_scatter_add(
                            dense_d[:], out_sb[:, SUBS // 2:SUBS, :],
                            idxs_tok[:, g * GCOLS + GCOLS // 2:
                                     (g + 1) * GCOLS],
                            TOKG // 2, TOKG // 2, H)
                    else:
                        nc.gpsimd.dma_scatter_add(
                            dense_d[:], out_sb[:],
                            idxs_tok[:, g * GCOLS:(g + 1) * GCOLS],
                            TOKG, TOKG, H)

            # ---------------- combine ----------------
            nc.gpsimd.collective_compute(
                "ReduceScatter", Alu.add, replica_groups=groups,
                ins=[dense_d[:]], outs=[rs_out_d[:]],
            )
            nc.sync.dma_start(
                y_d[:].rearrange("(p a) h -> p a h", p=128),
                rs_out_d[:].rearrange("(p a) h -> p a h", p=128))

    if do_compile:
        nc.compile()
        _split_excess_waits(nc)
    return nc


# ---------------------------------------------------------------------------
# host-side sharding + execution
# ---------------------------------------------------------------------------

def _pcf(w):
    """[CH*128, F] -> host pre-swizzle to the [128, CH*F] (p, c, f) layout."""
    ch = w.shape[0] // 128
    return np.ascontiguousarray(
        w.reshape(ch, 128, w.shape[1]).transpose(1, 0, 2).reshape(128, -1))


def make_in_maps(x, Wg, W1, b1, W2, b2, N=8192, H=1024):
    xt = np.ascontiguousarray(x.reshape(N, H).astype(np.float32))
    x_bf = xt.astype(ml_dtypes.bfloat16)
    P = N // N_CORES
    in_maps = []
    for c in range(N_CORES):
        shard = xt[c * P:(c + 1) * P, :]
        in_maps.append({
            "xT_s": np.ascontiguousarray(shard.T),
            "x_bf": x_bf,
            "Wg": np.ascontiguousarray(Wg.astype(np.float32)),
            "W1": _pcf(W1[c].astype(ml_dtypes.bfloat16)),
            "W2": _pcf(W2[c].astype(ml_dtypes.bfloat16)),
            "b1": np.ascontiguousarray(b1[c].reshape(1, -1).astype(np.float32)),
            "b2": np.ascontiguousarray(b2[c].reshape(1, -1).astype(np.float32)),
        })
    return in_maps


_NC_CACHE = {}


def kernel(x, Wg, W1, b1, W2, b2):
    x = np.asarray(x)
    B, L, H = x.shape
    N = B * L
    FF = W1.shape[2]
    key = (N, H, FF)
    if key not in _NC_CACHE:
        _NC_CACHE[key] = build_moe_nc(N=N, H=H, FF=FF)
    nc = _NC_CACHE[key]
    in_maps = make_in_maps(np.asarray(x), np.asarray(Wg), np.asarray(W1),
                           np.asarray(b1), np.asarray(W2), np.asarray(b2),
                           N=N, H=H)
    from concourse.bass_utils import run_bass_kernel_spmd
    res = run_bass_kernel_spmd(nc, in_maps, core_ids=list(range(N_CORES)),
                               trace=False)
    out = np.concatenate([res.results[c]["y"] for c in range(N_CORES)], axis=0)
    return out.reshape(B, L, H).astype(np.float32)


# revision 51
# speedup vs baseline: 1.1395x; 1.1395x over previous
"""Expert-choice MoE kernel for 8 Trainium2 NeuronCores (Bacc/Tile).

Distribution: expert-parallel, one expert per core.
  - gate: each core computes fp32 scores z = x_shard @ Wg for its 1/8 token
    shard (pre-swizzled so the wrapped-16 view loads contiguously later),
    AllToAll -> each core holds the full (N,) score row of ITS expert.
  - top-k (k=2048 of N=8192): exact fp32 threshold via a parallel-64
    candidate search (each partition holds HALF the logits; one
    tensor_scalar/accum count per round + a fold matmul merges the
    halves; 5 rounds of 64x narrowing from [-2,2) = 3.7e-9 resolution
    < fp32 ulp at the threshold), then ONE gpsimd sparse_gather
    compaction of fp32-packed (idx*2048 + 2*q10(gate)+1) values, exact
    in 24 bits (pad columns absorb score ties at the threshold).
  - dispatch: ONE dma_gather(transpose=True) per 512-token group pulls the
    selected rows from HBM already transposed to [h, tok] bf16 layout.
  - expert FFN in bf16 (fp32 accumulation), erf-Gelu on the scalar engine,
    fp32 gate multiply on the bf16 output. W2 runs fc-outer so each hid
    stationary serves two 512-wide matmuls.
  - combine: ONE dma_scatter_add (SDMA CCE add) per group into a zeroed
    bf16 (N, H) dense buffer, ReduceScatter (add, bf16) across the 8
    cores; y is emitted bf16 and upcast on the host.

Built on Bacc (not raw Bass): Bacc.compile() runs insert_library_loads
and codegen_inst_isa_subclasses, which this walrus build needs to accept
the sparse_gather/dma_gather/dma_scatter_add Pool-ucode instructions.
"""

import sys

for _p in ("/opt/trn_rl_repo",):
    if _p not in sys.path:
        sys.path.insert(0, _p)

import numpy as np
import ml_dtypes

import concourse.bass as bass
import concourse.bacc as bacc
import concourse.mybir as mybir
import concourse.tile as tile
from concourse.bass import _add_dep_helper

# ---------------------------------------------------------------------------
# Patch: this walrus build rejects >1 sync-wait on the SP Drain that
# TileContext emits at kernel exit. Split the global-clock waits across
# several drains (1 wait each).
# ---------------------------------------------------------------------------
from concourse.vector_clock import ScopedClock

_MAX_DRAIN_WAITS = 1


def _patched_drain_and_barrier(self, tick_clock, wait_clock):
    nc = self.nc
    probe = nc.sync.drain()
    wait_clock.add_sem_waits(probe.ins, ScopedClock({None: tick_clock.global_clock}))
    si = probe.ins.sync_info
    waits = list(si.on_wait or []) if si is not None else []
    if len(waits) > _MAX_DRAIN_WAITS:
        probe.ins.sync_info = mybir.SyncInfo(
            on_wait=waits[:_MAX_DRAIN_WAITS],
            on_update=list(si.on_update or []),
        )
        for i in range(_MAX_DRAIN_WAITS, len(waits), _MAX_DRAIN_WAITS):
            extra = nc.sync.drain()
            extra.ins.sync_info = mybir.SyncInfo(
                on_wait=waits[i : i + _MAX_DRAIN_WAITS], on_update=[]
            )
    nc.all_engine_barrier()
    assert self.sems is not None
    popped = nc._tile_sem_poison_stack.pop()
    assert popped is self._sem_poison
    nc.clear_and_free_semaphores(list(self.sems.allocated().values()))
    nc.all_engine_barrier()


tile.TileContext._drain_and_barrier = _patched_drain_and_barrier

_WSPLIT_LIMIT = 1
_wsplit_ctr = [0]


def _split_excess_waits(nc, limit=_WSPLIT_LIMIT):
    """This walrus build encodes at most `limit` sync-wait commands per
    instruction; hoist excess waits onto same-engine Drain instructions
    inserted immediately before (per-engine streams execute in order)."""
    f = nc.m.functions[0]
    for b in f.blocks:
        insts = b.instructions
        out = []
        changed = False
        for inst in insts:
            si = getattr(inst, "sync_info", None)
            waits = list(si.on_wait or []) if si is not None else []
            eng = getattr(inst, "engine", None)
            if len(waits) > limit and eng is not None and \
                    eng != mybir.EngineType.Unassigned:
                keep = waits[-limit:]
                extra = waits[:-limit]
                for i in range(0, len(extra), limit):
                    d = mybir.InstDrain(
                        name=f"WSPLIT-{_wsplit_ctr[0]}", ins=[], outs=[])
                    _wsplit_ctr[0] += 1
                    d.engine = eng
                    d.sync_info = mybir.SyncInfo(
                        on_wait=extra[i:i + limit], on_update=[])
                    out.append(d)
                    nc.register_instruction(d, overwrite=True)
                inst.sync_info = mybir.SyncInfo(
                    on_wait=keep, on_update=list(si.on_update or []))
                changed = True
            out.append(inst)
        if changed:
            b.instructions = out

dt = mybir.dt
Alu = mybir.AluOpType
Act = mybir.ActivationFunctionType

N_CORES = 8

FULL = dict(N=8192, H=1024, FF=4096, E=8, K=2048)


def build_moe_nc(N=8192, H=1024, FF=4096, E=8, K=2048, TOKG=512, act=None,
                 do_compile=True, skip_sg=False):
    """Build the SPMD Bacc program (same program on all 8 cores)."""
    assert E == N_CORES
    P = N // N_CORES          # tokens per shard
    HC = H // 128             # h chunks
    FC = FF // 128            # ff chunks
    NG = K // TOKG            # token groups
    SUBS = TOKG // 128        # 128-token subtiles per group
    NCOLS = K // 128          # compact cols in [128, NCOLS] layout
    W16 = N // 16             # free size of the [16, W16] wrapped layout
    K16 = K // 16             # compact cols in [16, K16] wrapped layout
    GCOLS = TOKG // 16        # idx cols consumed per group
    SGPAD = 64                # tie-absorbing pad columns for sparse_gather
    TRASH = P                 # local-combine trash row for non-local tokens
    assert K % TOKG == 0 and TOKG % 128 == 0 and P % 128 == 0
    assert K16 + SGPAD <= 512  # sparse_gather output limit
    if act is None:
        act = Act.Gelu

    nc = bacc.Bacc(None, target_bir_lowering=False, debug=False,
                   num_devices=N_CORES)

    # ---- I/O ----
    xT_s = nc.dram_tensor("xT_s", [H, P], dt.float32, kind="ExternalInput")
    x_bf = nc.dram_tensor("x_bf", [N, H], dt.bfloat16, kind="ExternalInput")
    Wg_d = nc.dram_tensor("Wg", [H, E], dt.float32, kind="ExternalInput")
    # W1/W2 uploaded HOST-PRE-SWIZZLED to the [p, c, f] SBUF layout the
    # gather-transpose contraction needs (h = c*128 + p): the device-side
    # load is then partition-contiguous — 128 big descriptors instead of
    # 1024, cutting ~40us of descriptor-gen off the sync queue.
    W1_d = nc.dram_tensor("W1", [128, HC * FF], dt.bfloat16,
                          kind="ExternalInput")
    W2_d = nc.dram_tensor("W2", [128, FC * H], dt.bfloat16,
                          kind="ExternalInput")
    b1_d = nc.dram_tensor("b1", [1, FF], dt.float32, kind="ExternalInput")
    b2_d = nc.dram_tensor("b2", [1, H], dt.float32, kind="ExternalInput")
    # per-core shard offset (c*P) for remapping global token ids to the
    # local dense buffer; [16,1] so it is a per-partition scalar operand
    # y is emitted in bf16 (the combine is bf16 anyway); the host upcasts
    y_d = nc.dram_tensor("y", [P, H], dt.bfloat16, kind="ExternalOutput")

    # ---- internal DRAM ----
    z_loc_d = nc.dram_tensor("z_loc", [E, P], dt.float32)
    z_e_d = nc.dram_tensor("z_e", [N_CORES, P], dt.float32)
    g_dram = nc.dram_tensor("g_dram", [K], dt.float32)
    dense_d = nc.dram_tensor("dense", [N, H], dt.bfloat16)
    rs_out_d = nc.dram_tensor("rs_out", [P, H], dt.bfloat16)

    groups = [list(range(N_CORES))]

    with tile.TileContext(nc) as tc:
        with (
            tc.tile_pool(name="const", bufs=1) as const_pool,
            tc.tile_pool(name="w", bufs=1) as w_pool,
            tc.tile_pool(name="psum1", bufs=2, space="PSUM") as psum1_pool,
            tc.tile_pool(name="psum2", bufs=2, space="PSUM") as psum2_pool,
        ):
            # ---------------- persistent constants ----------------
            ones1 = const_pool.tile([1, 128], dt.float32)
            nc.vector.memset(ones1[:], 1.0)
            ones64 = const_pool.tile([64, 128], dt.float32)
            nc.vector.memset(ones64[:], 1.0)

            # b2 broadcast [128, H] (constant along tokens)
            b2_sb = const_pool.tile([1, H], dt.float32)
            nc.sync.dma_start(b2_sb[:], b2_d[:])
            b2_ps = psum2_pool.tile([128, H], dt.float32, tag="ps2")
            for hh in range(0, H, 512):
                nc.tensor.matmul(b2_ps[:, hh:hh + 512], ones1[:],
                                 b2_sb[:, hh:hh + 512], start=True, stop=True)
            b2_bcast = const_pool.tile([128, H], dt.float32)
            nc.vector.tensor_copy(b2_bcast[:], b2_ps[:])

            # b1 per-partition [128, FC]
            b1_pp = const_pool.tile([128, FC], dt.float32)
            nc.sync.dma_start(
                b1_pp[:], b1_d[:].rearrange("o (c p) -> (o p) c", p=128))

            # persistent routing outputs (filled by the gate phase)
            idxs_tok = const_pool.tile([128, K16], dt.int16)
            g_pp = const_pool.tile([128, NCOLS], dt.float32)

            # ================= gate phase (scoped pool) ================
            # Emitted BEFORE the (much larger) weight DMAs so the
            # scheduler gives the latency-critical gate inputs DMA priority.
            sg_inst = None
            with (
                tc.tile_pool(name="gate", bufs=1) as gate_pool,
                tc.tile_pool(name="small", bufs=2) as small_pool,
            ):
                # h mapped as p*HC + c so each partition reads ONE
                # contiguous block per load (128 big DMA descriptors).
                # Loaded as FOUR h-quarters through a 2-buffer tag rotation
                # (16KB total; the slots are later reused by the z_bcast
                # halves) so the loads pipeline with the z matmuls; the z
                # matmul accumulates through all quarters before stopping.
                HQ = HC // 4
                wg_sb = gate_pool.tile([128, HC, E], dt.float32)
                nc.sync.dma_start(
                    wg_sb[:], Wg_d[:].rearrange("(p c) e -> p c e", p=128))

                # z_sb_loc rows are written PRE-SWIZZLED (token u stored at
                # column (u%16)*64 + u//16) so that after the AllToAll the
                # wrapped-16 [16, W16] view is a contiguous-stride load.
                z_sb_loc = gate_pool.tile([E, P], dt.float32)
                z_loc_sw = z_sb_loc[:].rearrange("e (r w) -> e w r", r=16)
                z_ps_l = []
                for t0 in range(0, P, 512):
                    z_ps = psum1_pool.tile([E, 512], dt.float32,
                                           name=f"z_ps{t0}", tag="ps1")
                    z_ps_l.append(z_ps)
                for qf in range(4):
                    xT_sb = gate_pool.tile([128, HQ, P], dt.float32,
                                           name=f"xT{qf}", tag="xT_sb",
                                           bufs=2)
                    nc.sync.dma_start(
                        xT_sb[:],
                        xT_s[:].rearrange("(p c) t -> p c t", p=128)
                        [:, qf * HQ:(qf + 1) * HQ, :])
                    for t0 in range(0, P, 512):
                        z_ps = z_ps_l[t0 // 512]
                        for ci in range(HQ):
                            nc.tensor.matmul(
                                z_ps[:], wg_sb[:, qf * HQ + ci, :],
                                xT_sb[:, ci, t0:t0 + 512],
                                start=(qf == 0 and ci == 0),
                                stop=(qf == 3 and ci == HQ - 1))
                for t0 in range(0, P, 512):
                    nc.vector.tensor_copy(
                        z_loc_sw[:, t0 // 16:(t0 + 512) // 16, :],
                        z_ps_l[t0 // 512][:])
                nc.sync.dma_start(z_loc_d[:], z_sb_loc[:])

                # core c receives every shard's scores for expert c
                nc.gpsimd.collective_compute(
                    "AllToAll", Alu.bypass, replica_groups=groups,
                    ins=[z_loc_d[:]], outs=[z_e_d[:]],
                )

                # wrapped-16 view (token j at [j%16, j//16]); contiguous
                # 64-element runs thanks to the sender-side swizzle
                # (slot shared with the now-dead z_sb_loc)
                z16 = gate_pool.tile([16, W16], dt.float32, tag="z_sb_loc")
                z16_inst = nc.sync.dma_start(
                    z16[:].rearrange("r (q w) -> r q w", q=E),
                    z_e_d[:].rearrange("q (r w) -> r q w", r=16))

                # compaction inputs that do not depend on the threshold —
                # emitted first so they overlap the A2A / bisection
                ids16 = gate_pool.tile([16, W16], dt.int32)
                nc.gpsimd.iota(ids16[:], pattern=[[16, W16]], base=0,
                               channel_multiplier=1)
                idf16 = gate_pool.tile([16, W16], dt.float32)
                nc.vector.tensor_copy(idf16[:], ids16[:])
                sig16 = gate_pool.tile([16, W16], dt.float32)
                nc.scalar.activation(sig16[:], z16[:], Act.Sigmoid)

                # ---- parallel-64 search for the k-th largest logit ----
                # Partitions p and p+64 each hold HALF of the N logits
                # (z_bcast2[p] = z16 half p//64); round r tests the 64
                # candidates base + (p%64)*step_r at once (one
                # tensor_scalar with accum_out gives per-partition partial
                # counts; partition-sliced add merges the halves).
                # base' = base + (j*-1)*step with j* = #candidates whose
                # count >= K keeps count(z >= base) >= K invariant; each
                # round shrinks the bracket 64x. Start [-2, 2) (logits are
                # N(0, ~0.64); only the THRESHOLD must lie inside the
                # bracket, and it is the 75th percentile ~= +0.43).
                # 5 rounds -> 3.7e-9 < fp32 ulp of the logits.
                # eqm[k', k*64+m] = (k==k'): selector for the broadcast
                # (iota shares the later-used scr8 slot — same 4KB)
                eqm_i = gate_pool.tile([16, 16 * 64], dt.int32, tag="scr8")
                nc.gpsimd.iota(eqm_i[:], pattern=[[1, 16], [0, 64]], base=0,
                               channel_multiplier=-1)
                eqm = gate_pool.tile([16, 16 * 64], dt.float32)
                nc.vector.tensor_scalar(eqm[:], eqm_i[:], 0, None,
                                        op0=Alu.is_equal)
                iota_p = gate_pool.tile([128, 1], dt.int32)
                nc.gpsimd.iota(iota_p[:], pattern=[[1, 1]], base=0,
                               channel_multiplier=1)
                iota_p64 = gate_pool.tile([128, 1], dt.int32)
                nc.vector.tensor_scalar(iota_p64[:], iota_p[:], 63, None,
                                        op0=Alu.bitwise_and)
                iota_pf = gate_pool.tile([128, 1], dt.float32)
                nc.vector.tensor_copy(iota_pf[:], iota_p64[:])
                # fold[p, j] = (p % 64 == j): folds the two partition
                # halves' partial counts with one matmul (a cross-base-
                # partition tensor_tensor is rejected by the verifier)
                fold_i = gate_pool.tile([128, 64], dt.int32)
                nc.gpsimd.iota(fold_i[:], pattern=[[1, 64]], base=0,
                               channel_multiplier=-1)
                nc.vector.tensor_scalar(fold_i[:], fold_i[:], 128, None,
                                        op0=Alu.add)
                nc.vector.tensor_scalar(fold_i[:], fold_i[:], 63, None,
                                        op0=Alu.bitwise_and)
                fold = gate_pool.tile([128, 64], dt.float32)
                nc.vector.tensor_scalar(fold[:], fold_i[:], 0, None,
                                        op0=Alu.is_equal)

                HW16 = W16 // 2   # 256: half of the wrapped free dim
                # two 8KB halves reusing the rotating xT slots
                zb_halves = []
                for h in range(2):
                    zbh = gate_pool.tile([128, 8, HW16], dt.float32,
                                         name=f"zb{h}", tag="xT_sb", bufs=2)
                    zb_halves.append(zbh)
                for k in range(16):
                    z_bcast = zb_halves[k // 8]
                    zb_ps = psum1_pool.tile([128, HW16], dt.float32,
                                            tag="ps1")
                    nc.tensor.matmul(zb_ps[0:64, :],
                                     eqm[:, k * 64:(k + 1) * 64],
                                     z16[:, 0:HW16],
                                     start=True, stop=True)
                    nc.tensor.matmul(zb_ps[64:128, :],
                                     eqm[:, k * 64:(k + 1) * 64],
                                     z16[:, HW16:W16],
                                     start=True, stop=True)
                    if k % 2 == 0:
                        nc.vector.tensor_copy(z_bcast[:, k % 8, :], zb_ps[:])
                    else:
                        nc.scalar.activation(z_bcast[:, k % 8, :], zb_ps[:],
                                             Act.Copy)

                kf = float(K)
                rounds = 5
                base = small_pool.tile([128, 1], dt.float32, tag="base")
                nc.vector.memset(base[:], -2.0)
                step = 4.0 / 64.0
                scr8 = gate_pool.tile([128, 8 * HW16], dt.uint8)
                for _ in range(rounds):
                    cand = small_pool.tile([128, 1], dt.float32, tag="cand")
                    nc.vector.scalar_tensor_tensor(
                        cand[:], iota_pf[:], step, base[:],
                        op0=Alu.mult, op1=Alu.add)
                    partA = small_pool.tile([128, 1], dt.float32, tag="pA")
                    nc.vector.tensor_scalar(
                        scr8[:],
                        zb_halves[0][:].rearrange("p a b -> p (a b)"),
                        cand[:, :1], None, op0=Alu.is_ge, op1=Alu.add,
                        accum_out=partA[:])
                    partB = small_pool.tile([128, 1], dt.float32, tag="pB")
                    nc.vector.tensor_scalar(
                        scr8[:],
                        zb_halves[1][:].rearrange("p a b -> p (a b)"),
                        cand[:, :1], None, op0=Alu.is_ge, op1=Alu.add,
                        accum_out=partB[:])
                    part = small_pool.tile([128, 1], dt.float32, tag="part")
                    nc.vector.tensor_tensor(part[:], partA[:], partB[:],
                                            op=Alu.add)
                    cnt_ps = psum1_pool.tile([64, 1], dt.float32, tag="ps1")
                    nc.tensor.matmul(cnt_ps[:], fold[:], part[:],
                                     start=True, stop=True)
                    geK = small_pool.tile([64, 1], dt.float32, tag="geK")
                    nc.vector.tensor_scalar(geK[:], cnt_ps[:], kf, None,
                                            op0=Alu.is_ge)
                    js_ps = psum1_pool.tile([128, 1], dt.float32, tag="ps1")
                    nc.tensor.matmul(js_ps[:], ones64[:], geK[:],
                                     start=True, stop=True)
                    # base' = fl((j*-1)*step + base) — the SAME rounding
                    # path as the tested candidate, so the count(z >= base)
                    # >= K invariant holds bit-exactly.
                    jm1 = small_pool.tile([128, 1], dt.float32, tag="jm1")
                    nc.vector.tensor_scalar(jm1[:], js_ps[:], -1.0, None,
                                            op0=Alu.add)
                    nb = small_pool.tile([128, 1], dt.float32, tag="base")
                    nc.vector.scalar_tensor_tensor(
                        nb[:], jm1[:], step, base[:],
                        op0=Alu.mult, op1=Alu.add)
                    base = nb
                    step /= 64.0
                lo = base

                # ---- selection mask + ONE packed compaction ----
                # packed = idx*2048 + (2*q10(sig) + 1): 24 bits, exact fp32.
                # Unselected slots get -1.0 (negative -> dropped by
                # sparse_gather). Output padded by SGPAD columns so fp32
                # score ties at the threshold overflow into the pad.
                sel16 = gate_pool.tile([16, W16], dt.uint8)
                nc.vector.tensor_scalar(sel16[:], z16[:], lo[:16, :1], None,
                                        op0=Alu.is_ge)
                q1 = gate_pool.tile([16, W16], dt.float32)
                nc.vector.tensor_scalar(q1[:], sig16[:], 1023.0, None,
                                        op0=Alu.mult)
                q2 = gate_pool.tile([16, W16], dt.int32)
                nc.vector.tensor_copy(q2[:], q1[:])
                nc.vector.tensor_scalar(q2[:], q2[:], 1, 1,
                                        op0=Alu.logical_shift_left,
                                        op1=Alu.bitwise_or)
                q4 = gate_pool.tile([16, W16], dt.float32, tag="ids16")
                nc.vector.tensor_copy(q4[:], q2[:])
                packed = gate_pool.tile([16, W16], dt.float32, tag="q2")
                nc.vector.scalar_tensor_tensor(
                    packed[:], idf16[:], 2048.0, q4[:],
                    op0=Alu.mult, op1=Alu.add)
                pneg = gate_pool.tile([16, W16], dt.float32, tag="q1")
                nc.vector.memset(pneg[:], -1.0)
                nc.vector.copy_predicated(pneg[:], sel16[:], packed[:])
                pc = gate_pool.tile([16, K16 + SGPAD], dt.float32)
                nf1 = gate_pool.tile([1, 1], dt.uint32)
                if not skip_sg:
                    sg_inst = nc.gpsimd.sparse_gather(pc[:], pneg[:],
                                                      num_found=nf1[:])
                else:
                    fake = gate_pool.tile([16, K16 + SGPAD], dt.int32)
                    nc.gpsimd.iota(fake[:], pattern=[[16, K16 + SGPAD]],
                                   base=0, channel_multiplier=1)
                    nc.vector.tensor_scalar(fake[:], fake[:], 11, 1025,
                                            op0=Alu.logical_shift_left,
                                            op1=Alu.bitwise_or)
                    nc.vector.tensor_copy(pc[:], fake[:])

                # unpack: idx = packed >> 11, gq = packed & 2047 (int-exact)
                pci = gate_pool.tile([16, K16], dt.int32)
                nc.vector.tensor_copy(pci[:], pc[:, :K16])
                idx_i32 = gate_pool.tile([16, K16], dt.int32)
                nc.vector.tensor_scalar(idx_i32[:], pci[:], 11, None,
                                        op0=Alu.arith_shift_right)
                idc_i16 = gate_pool.tile([16, K16], dt.int16)
                nc.vector.tensor_copy(idc_i16[:], idx_i32[:])
                gq_i32 = gate_pool.tile([16, K16], dt.int32)
                nc.vector.tensor_scalar(gq_i32[:], pci[:], 2047, None,
                                        op0=Alu.bitwise_and)
                gqf = gate_pool.tile([16, K16], dt.float32)
                nc.vector.tensor_copy(gqf[:], gq_i32[:])
                gc = gate_pool.tile([16, K16], dt.float32)
                nc.vector.tensor_scalar(gc[:], gqf[:], 1.0, 1.0 / 2046.0,
                                        op0=Alu.subtract, op1=Alu.mult)

                # idxs replicated to all 8 core blocks (for gather/scatter)
                for b in range(8):
                    nc.sync.dma_start(idxs_tok[16 * b:16 * (b + 1), :],
                                      idc_i16[:])

                # gate values to per-partition [128, NCOLS] via DRAM bounce
                nc.sync.dma_start(
                    g_dram[:].rearrange("(c r) -> r c", r=16), gc[:])
                nc.sync.dma_start(
                    g_pp[:], g_dram[:].rearrange("(q p) -> p q", p=128))

                # zero the dense combine buffer (16.8MB; each partition
                # writes contiguous 4-row (8KB) runs). SWDGE (gpsimd)
                # keeps the HWDGE rings clear for the latency-critical
                # gate-phase traffic (z store -> A2A).
                zero_sb = gate_pool.tile([128, 4, H], dt.bfloat16)
                nc.vector.memset(zero_sb[:], 0.0)
                for i in range(N // 512):
                    nc.sync.dma_start(
                        dense_d[512 * i:512 * (i + 1), :].rearrange(
                            "(p c) h -> p c h", p=128),
                        zero_sb[:])

            # ------- weights (fill DMA idle time of the gate phase;
            # emitted after it so the gate loads win priority) ---
            # Held back until the z16 load lands: 16MB of weight traffic
            # issued too early floods the SDMA rings and delays the tiny
            # gate-phase transfers (z store -> A2A -> z16) by 10-15us.
            w1_sb = w_pool.tile([128, HC, FF], dt.bfloat16)
            w1i = nc.sync.dma_start(
                w1_sb[:], W1_d[:].rearrange("p (c f) -> p c f", c=HC))
            _add_dep_helper(w1i.ins, z16_inst.ins,
                            reason="keep SDMA rings clear for gate phase")
            # w2 is not needed until the first W2 matmul (~245us); held
            # until the FIRST GATHER has issued so only w1's 8MB rides the
            # rings during the gate tail (sparse_gather -> gather chain).
            w2_sb = w_pool.tile([128, FC, H], dt.bfloat16)
            w2i = nc.sync.dma_start(
                w2_sb[:], W2_d[:].rearrange("p (c h) -> p c h", c=FC))
            _add_dep_helper(w2i.ins, z16_inst.ins,
                            reason="keep SDMA rings clear for gate phase")

            # ================= FFN phase ================
            with (
                tc.tile_pool(name="ex", bufs=2) as ex_pool,
                tc.tile_pool(name="hid", bufs=1) as hid_pool,
                tc.tile_pool(name="out", bufs=2) as out_pool,
            ):
                ex_tiles = {}

                def issue_gather(g):
                    ex_T = ex_pool.tile([128, HC, TOKG], dt.bfloat16,
                                        tag="ex")
                    g_inst = nc.gpsimd.dma_gather(
                        ex_T[:], x_bf[:],
                        idxs_tok[:, g * GCOLS:(g + 1) * GCOLS],
                        TOKG, TOKG, H, transpose=True)
                    if g == 0 and sg_inst is not None:
                        _add_dep_helper(g_inst.ins, sg_inst.ins,
                                        reason="keep sparse_gather lib "
                                               "phase before mlp phase")
                    if g == 0:
                        # release the 8MB w2 stream only after the gate
                        # tail's latency-critical transfers have issued
                        _add_dep_helper(w2i.ins, g_inst.ins,
                                        reason="w2 stream after gather 0")
                    ex_tiles[g] = ex_T

                issue_gather(0)
                for g in range(NG):
                    if g + 1 < NG:
                        issue_gather(g + 1)
                    ex_T = ex_tiles.pop(g)

                    hid_sb = hid_pool.tile([128, FC, TOKG], dt.bfloat16,
                                           tag="hid")
                    for fc in range(FC):
                        ps1 = psum1_pool.tile([128, TOKG], dt.float32,
                                              tag="ps1")
                        for ci in range(HC):
                            nc.tensor.matmul(
                                ps1[:], w1_sb[:, ci, fc * 128:(fc + 1) * 128],
                                ex_T[:, ci, :],
                                start=(ci == 0), stop=(ci == HC - 1))
                        nc.scalar.activation(hid_sb[:, fc, :], ps1[:], act,
                                             bias=b1_pp[:, fc:fc + 1])

                    out_sb = out_pool.tile([128, SUBS, H], dt.bfloat16,
                                           tag="osb")
                    for s in range(SUBS):
                        col = g * SUBS + s
                        pso = psum2_pool.tile([128, H], dt.float32, tag="ps2")
                        # fc-outer: each hid stationary serves both hh
                        # halves (one PE weight swap per TWO matmuls)
                        for fc in range(FC):
                            lhs = hid_sb[:, fc, s * 128:(s + 1) * 128]
                            nc.tensor.matmul(
                                pso[:, 0:512], lhs, w2_sb[:, fc, 0:512],
                                start=(fc == 0), stop=(fc == FC - 1))
                            nc.tensor.matmul(
                                pso[:, 512:1024], lhs, w2_sb[:, fc, 512:1024],
                                start=(fc == 0), stop=(fc == FC - 1))
                        nc.vector.tensor_tensor(out_sb[:, s, :], pso[:],
                                                b2_bcast[:], op=Alu.add)
                        nc.vector.tensor_scalar(out_sb[:, s, :],
                                                out_sb[:, s, :],
                                                g_pp[:, col:col + 1], None,
                                                op0=Alu.mult)
                        if g == NG - 1 and s == SUBS // 2 - 1:
                            # last group: scatter the first token-half now
                            # so its ~12us DMA data phase hides under the
                            # remaining W2 compute and the ReduceScatter
                            # triggers earlier
                            nc.gpsimd.dma_scatter_add(
                                dense_d[:], out_sb[:, 0:SUBS // 2, :],
                                idxs_tok[:, g * GCOLS:
                                         g * GCOLS + GCOLS // 2],
                                TOKG // 2, TOKG // 2, H)

                    if g == NG - 1:
                        nc.gpsimd.dma_scatter_add(
                            dense_d[:], out_sb[:, SUBS // 2:SUBS, :],
                            idxs_tok[:, g * GCOLS + GCOLS // 2:
                                     (g + 1) * GCOLS],
                            TOKG // 2, TOKG // 2, H)
                    else:
                        nc.gpsimd.dma_scatter_add(
                            dense_d[:], out_sb[:],
                            idxs_tok[:, g * GCOLS:(g + 1) * GCOLS],
                            TOKG, TOKG, H)

            # ---------------- combine ----------------
            nc.gpsimd.collective_compute(
                "ReduceScatter", Alu.add, replica_groups=groups,
                ins=[dense_d[:]], outs=[rs_out_d[:]],
            )
            nc.sync.dma_start(
                y_d[:].rearrange("(p a) h -> p a h", p=128),
                rs_out_d[:].rearrange("(p a) h -> p a h", p=128))

    if do_compile:
        nc.compile()
        _split_excess_waits(nc)
    return nc


# ---------------------------------------------------------------------------
# host-side sharding + execution
# ---------------------------------------------------------------------------

def _pcf(w):
    """[CH*128, F] -> host pre-swizzle to the [128, CH*F] (p, c, f) layout."""
    ch = w.shape[0] // 128
    return np.ascontiguousarray(
        w.reshape(ch, 128, w.shape[1]).transpose(1, 0, 2).reshape(128, -1))


def make_in_maps(x, Wg, W1, b1, W2, b2, N=8192, H=1024):
    xt = np.ascontiguousarray(x.reshape(N, H).astype(np.float32))
    x_bf = xt.astype(ml_dtypes.bfloat16)
    P = N // N_CORES
    in_maps = []
    for c in range(N_CORES):
        shard = xt[c * P:(c + 1) * P, :]
        in_maps.append({
            "xT_s": np.ascontiguousarray(shard.T),
            "x_bf": x_bf,
            "Wg": np.ascontiguousarray(Wg.astype(np.float32)),
            "W1": _pcf(W1[c].astype(ml_dtypes.bfloat16)),
            "W2": _pcf(W2[c].astype(ml_dtypes.bfloat16)),
            "b1": np.ascontiguousarray(b1[c].reshape(1, -1).astype(np.float32)),
            "b2": np.ascontiguousarray(b2[c].reshape(1, -1).astype(np.float32)),
        })
    return in_maps


_NC_CACHE = {}


def kernel(x, Wg, W1, b1, W2, b2):
    x = np.asarray(x)
    B, L, H = x.shape
    N = B * L
    FF = W1.shape[2]
    key = (N, H, FF)
    if key not in _NC_CACHE:
        _NC_CACHE[key] = build_moe_nc(N=N, H=H, FF=FF)
    nc = _NC_CACHE[key]
    in_maps = make_in_maps(np.asarray(x), np.asarray(Wg), np.asarray(W1),
                           np.asarray(b1), np.asarray(W2), np.asarray(b2),
                           N=N, H=H)
    from concourse.bass_utils import run_bass_kernel_spmd
    res = run_bass_kernel_spmd(nc, in_maps, core_ids=list(range(N_CORES)),
                               trace=False)
    out = np.concatenate([res.results[c]["y"] for c in range(N_CORES)], axis=0)
    return out.reshape(B, L, H).astype(np.float32)
